# revision 81
# baseline (speedup 1.0000x reference)
"""Trainium2 Bass kernel for MultiHeadAttention (B=8, S=1024, D=1024, H=16, DK=DV=64).

Sharding: data-parallel over batch — each of the 8 NeuronCores computes one
full batch element (QKV projections, masked+scaled softmax attention, output
projection, LayerNorm). No collectives.

Per-core math (batch b), matmul datapath in bf16 (f32 PSUM accumulate):
  Qt = (Wq/8)^T Xq^T          [hd, s]   (head-dim-major / transposed)
  Kt = Wk^T Xk^T              [hd, s]
  V' = Xv Wv (+ ones col)     [s, h*65]
  scores^T = K_h Q_h^T        [k, q] per head
  t = scores^T * mT           (mT = (matrix * !mask).T; masked entries -> 0)
  p = exp(t)                  (masked entries become exp(0) = 1)
  ctx^T = V'_h^T (p^T - maskT)  [65, q]  (row 64 = corrected denominator;
                               the -maskT matmul accumulates into the same
                               PSUM group and exactly removes the masked
                               exp(0)=1 contributions)
  renorm: DVE reciprocal of the denom row -> rank-1 PE broadcast matmul ->
          ACT copy -> fused PSUM*recip eviction multiply on DVE (odd heads
          DMA-shift to partitions 64-127 via the idle Pool queue).  The five
          renorm stages are emitted one head-group late at fixed kc slots
          (RN_SLOTS) of the next group so no engine queue parks at its head
          waiting on an upstream stage.
  out = LN(Ctx Wfc / denom) * gamma + beta
"""
from contextlib import ExitStack

import numpy as np

import concourse.bass as bass
import concourse.bacc as bacc
import concourse.tile as tile
import concourse.mybir as mybir
from concourse.bass_utils import run_bass_kernel_spmd
from concourse.masks import make_identity

F32 = mybir.dt.float32
F32R = mybir.dt.float32r
BF16 = mybir.dt.bfloat16
AF = mybir.ActivationFunctionType
ALU = mybir.AluOpType

B, S, DM, H, DK = 8, 1024, 1024, 16, 64
P = 128
ST = S // P      # seq tiles (8)
DT = DM // P     # d_model tiles (8)
QB = 512         # q-block width in attention phase
NQB = S // QB
HPG = 2          # heads per PSUM group
SUB_HGS = 0      # head-groups whose mask correction runs as an in-place
                 # DVE p4+=maskT (bf16 2x mode) instead of a PE matmul;
                 # tuned so PE and DVE loads balance
SC_BUFS = 2      # scores psum depth (each tile holds both heads: 2 banks)
CTX_BUFS = 2     # ctx psum depth
T4_BUFS = 4      # t4 depth (DVE->ACT hop)
P4_BUFS = 4      # p4 depth (ACT->PE hop)
D_ILV = False    # interleave first-half phase D into the last q-block
RN_SLOTS = (0, 1, 3, 5, 7)  # kc slots (in the next group) where the five
                 # renorm stages fire: recip, rank1, copy, mul0, mul1.
                 # Wider spread beats (0,1,2,4,6) by ~6us; compressing to
                 # (0,1,2,3,4) loses ~12us — the DVE evict-muls must not
                 # bunch up behind the ACT copy
RN_PSUM_RB = False  # eviction muls read the recip broadcast straight from
                 # PSUM (skip the ACT copy hop) — compiler rejects; keep False
RENORM_MODE = "pe"  # "pe": rank-1 PE broadcast + ACT copy;  "dram": recip
                 # row round-trips through a DRAM scratch and broadcasts back
                 # via a stride-0-source DMA (all off the PE/ACT queues)
EXP_PAIR = False  # one ACT exp per TWO k-tiles — measured slower (coarser
                 # exp granularity delays the AV accumulation); keep False
LN_EPS = 1e-5


def r32(ap):
    return ap.bitcast(F32R)


def build_bass(apply_gamma_beta: bool, timing_reps: int = 0, phases: str = "ABCD",
               ablate: str = ""):
    nc = bacc.Bacc("TRN2", target_bir_lowering=False, debug=False,
                   enable_asserts=False, num_devices=8)

    timing = timing_reps > 0
    kind = "Internal" if timing else "ExternalInput"

    def dram_in(name, shape, dt):
        if timing:
            return nc.dram_tensor(name, shape, dt).ap()
        return nc.dram_tensor(name, shape, dt, kind="ExternalInput").ap()

    xq = dram_in("xq", [S, DM], BF16)
    xk = dram_in("xk", [S, DM], BF16)
    xv = dram_in("xv", [S, DM], BF16)
    mt = dram_in("mt", [S, S], BF16)         # (matrix*keep)^T [k,q]
    maskt = dram_in("maskt", [S, S], BF16)   # mask^T as float [k,q]
    wq = dram_in("wq", [DM, DM], BF16)       # pre-scaled by 1/sqrt(DK)
    wk = dram_in("wk", [DM, DM], BF16)
    wv = dram_in("wv", [DM, DM], BF16)
    wfc = dram_in("wfc", [DM, DM], BF16)
    rbd = nc.dram_tensor("rbd", [2, HPG * QB], F32).ap()  # renorm scratch
    ident_d = dram_in("ident", [P, P], BF16)  # host-provided identity: avoids
                                              # gpsimd make_identity (~8us/op
                                              # dispatch on HW) at startup
    if timing:
        out = nc.dram_tensor("out", [S, DM], BF16).ap()
        done = nc.dram_tensor("done", [1, 1], F32, kind="ExternalOutput").ap()
    else:
        out = nc.dram_tensor("out", [S, DM], BF16,
                             kind="ExternalOutput").ap()
    gamma = beta = None
    if apply_gamma_beta:
        gamma = dram_in("gamma", [DM], F32)
        beta = dram_in("beta", [DM], F32)

    mt_t = mt.rearrange("(t p) q -> p t q", p=P)
    maskt_t = maskt.rearrange("(t p) q -> p t q", p=P)

    with tile.TileContext(nc) as tc, ExitStack() as ctx:
        if timing:
            ctx.enter_context(tc.For_i(0, timing_reps, 1))
        const = ctx.enter_context(tc.tile_pool(name="const", bufs=1))
        eps_t = const.tile([P, 1], F32, tag="eps")
        nc.vector.memset(eps_t[:], LN_EPS)

        # Ctx^T lives through C+D; opened first so A-C pools can release
        ctx_pool = ctx.enter_context(tc.tile_pool(name="ctxp", bufs=1))
        ctx_sb = ctx_pool.tile([P, DT, S], BF16, tag="ctx")       # Ctx^T [hd, q]
        if "norenorm" in ablate:
            nc.vector.memset(ctx_sb[:], 0.0)  # keep phase D's reads legal
        # Wfc tiles also outlive C: DMAs are issued at the start of phase C
        # so phase D's matmuls never wait on them
        wfc_pool = ctx.enter_context(tc.tile_pool(name="wfc", bufs=1))
        wfc_tiles = [wfc_pool.tile([P, DM], BF16, tag=f"wfc{kc}",
                                   name=f"wfct{kc}")
                     for kc in range(DT)]

        # persistent across phases A-C (released before phase D)
        persist_cm = tc.tile_pool(name="persist", bufs=1)
        persist = persist_cm.__enter__()
        qt_sb = persist.tile([P, DT, S], BF16, tag="qt")          # Qt [hd, s]
        kt_sb = persist.tile([P, DT, S], BF16, tag="kt")          # Kt [hd, s]
        vp_sb = persist.tile([P, ST, H * 65], BF16, tag="vp")     # V' [s, h*65]
        vp_view = vp_sb.rearrange("p t (h d) -> p t h d", d=65)

        # ---------- Phase A: transposes + QKV projections ----------
        if "A" in phases:
          with tc.tile_pool(name="xrow", bufs=6) as xrow_pool, \
             tc.tile_pool(name="xT", bufs=3) as xT_pool, \
             tc.tile_pool(name="wload", bufs=6) as w_pool, \
             tc.tile_pool(name="aconst", bufs=1) as aconst, \
             tc.tile_pool(name="tp_psum", bufs=4, space="PSUM") as tp_psum, \
             tc.tile_pool(name="pj_psum", bufs=4, space="PSUM") as pj_psum:

            ident = aconst.tile([P, P], BF16, tag="ident")
            nc.sync.dma_start(ident[:], ident_d)

            def transpose_input(x_ap):
                """DRAM x [S, DM] -> SBUF x^T [P, DT, S] (partition=dm, free=s).

                Evictions ride DVE (idle in phase A; 2x bf16 mode) so ACT
                never sits between a transpose and the projection that
                consumes it."""
                xT = xT_pool.tile([P, DT, S], BF16, tag="xT")
                for i in range(ST):               # source s-tile
                    xrow = xrow_pool.tile([P, DM], BF16, tag="xrow")
                    nc.sync.dma_start(xrow[:], x_ap[i * P:(i + 1) * P, :])
                    for j0 in range(0, DT, 4):    # 4 dm-tiles per psum bank
                        ps = tp_psum.tile([P, 4, P], BF16, tag="tp")
                        for jj in range(4):
                            nc.tensor.matmul(ps[:, jj, :],
                                             xrow[:, (j0 + jj) * P:(j0 + jj + 1) * P],
                                             ident[:], is_transpose=True)
                        # strided evict: ps [P,4,P] -> xT[:, j0:j0+4, i*P:(i+1)*P]
                        nc.vector.tensor_copy(
                            xT[:, j0:j0 + 4, i * P:(i + 1) * P], ps[:])
                return xT

            def load_w_half(w_ap, half):
                """Stream one column-half of a weight matrix: [P, DT, DM/2]."""
                w_sb = w_pool.tile([P, DT, DM // 2], BF16, tag="w")
                nc.sync.dma_start(
                    w_sb[:],
                    w_ap.rearrange("(t p) n -> p t n", p=P)[
                        :, :, half * (DM // 2):(half + 1) * (DM // 2)])
                return w_sb

            def proj_T(w_sbs, xT, dst):
                """dst[hd, s] = W^T X^T : lhsT = W tiles [dm, hd], rhs = X^T [dm, s]."""
                for wh in range(2):               # W column halves
                    w_sb = w_sbs[wh]
                    for jm2 in range(DT // 2):    # hd out tiles in this half
                        jm = wh * (DT // 2) + jm2
                        pss = [pj_psum.tile([P, 512], F32, tag="pj",
                                            name=f"pj{jm}{sn}")
                               for sn in range(2)]
                        for kc in range(DT):      # stationary shared by 2 mms
                            for sn in range(2):
                                nc.tensor.matmul(
                                    pss[sn][:],
                                    w_sb[:, kc, jm2 * P:(jm2 + 1) * P],
                                    xT[:, kc, sn * 512:(sn + 1) * 512],
                                    start=(kc == 0), stop=(kc == DT - 1))
                        for sn in range(2):
                            # alternate evict engines: ACT and DVE both have
                            # slack under the PE-bound projections
                            ev = nc.scalar.copy if sn == 0 else \
                                nc.vector.tensor_copy
                            ev(dst[:, jm, sn * 512:(sn + 1) * 512],
                               pss[sn][:])

            # all three transposes are emitted first (xT triple-buffered):
            # the first projection then never bubbles on its own eviction
            # tail, and the weight DMAs stream under the transposes
            xkT = transpose_input(xk)
            wk_sbs = [load_w_half(wk, wh) for wh in range(2)]
            xqT = transpose_input(xq)
            wq_sbs = [load_w_half(wq, wh) for wh in range(2)]
            xvT = transpose_input(xv)
            wv_sbs = [load_w_half(wv, wh) for wh in range(2)]
            proj_T(wk_sbs, xkT, kt_sb)
            proj_T(wq_sbs, xqT, qt_sb)

            # V projection: natural [s, hd]; lhsT = Xv^T tiles, rhs = Wv halves
            for jm in range(ST):                  # s out tile
                pss = [pj_psum.tile([P, 512], F32, tag="pj", name=f"pv{jm}{wh}")
                       for wh in range(2)]
                for kc in range(DT):              # stationary shared by 2 mms
                    for wh in range(2):
                        nc.tensor.matmul(
                            pss[wh][:],
                            xvT[:, kc, jm * P:(jm + 1) * P],
                            wv_sbs[wh][:, kc, :],
                            start=(kc == 0), stop=(kc == DT - 1))
                for wh in range(2):
                    ev = nc.scalar.copy if wh == 0 else nc.vector.tensor_copy
                    ev(vp_view[:, jm, wh * 8:(wh + 1) * 8, 0:64],
                       pss[wh].rearrange("p (h d) -> p h d", d=64))
            nc.vector.memset(vp_view[:, :, :, 64:65], 1.0)

        if True:

            # ---------- Phase C: attention ----------
            if "C" in phases:
              with tc.tile_pool(name="mstream", bufs=1) as m_pool, \
                 tc.tile_pool(name="mask", bufs=1) as mask_pool, \
                 tc.tile_pool(name="att", bufs=T4_BUFS) as att_pool, \
                 tc.tile_pool(name="attp", bufs=P4_BUFS) as attp_pool, \
                 tc.tile_pool(name="cconst", bufs=1) as cconst, \
                 tc.tile_pool(name="rbp", bufs=2) as stg_pool, \
                 tc.tile_pool(name="dn", bufs=2) as dn_pool, \
                 tc.tile_pool(name="ln", bufs=2) as ln_pool, \
                 tc.tile_pool(name="lnstat", bufs=4) as stat_pool, \
                 tc.tile_pool(name="gb", bufs=1) as gb_pool, \
                 tc.tile_pool(name="sc_psum", bufs=SC_BUFS, space="PSUM") as sc_psum, \
                 tc.tile_pool(name="ctx_psum", bufs=CTX_BUFS, space="PSUM") as ctx_psum:
                maskt_sb = mask_pool.tile([P, ST, S], BF16, tag="maskt")
                nc.sync.dma_start(maskt_sb[:], maskt_t)
                if "D" in phases:
                    for kc in range(DT):
                        nc.sync.dma_start(wfc_tiles[kc][:],
                                          wfc[kc * P:(kc + 1) * P, :])
                # all-ones row at partition 64: stationary for the denominator
                # broadcast matmuls (rank-1 outer product with the recip row)
                ones64 = cconst.tile([P, P], F32, tag="ones64")
                nc.vector.memset(ones64[64:65, :], 1.0)
                gm = bt = None
                if apply_gamma_beta:
                    gm = gb_pool.tile([P, DM], F32, tag="gm")
                    bt = gb_pool.tile([P, DM], F32, tag="bt")
                    nc.sync.dma_start(gm[:], bass.AP(
                        tensor=gamma.tensor, offset=gamma.offset,
                        ap=[[0, P]] + list(gamma.ap)))
                    nc.sync.dma_start(bt[:], bass.AP(
                        tensor=beta.tensor, offset=beta.offset,
                        ap=[[0, P]] + list(beta.ap)))

                def emit_d_st(st):
                    # one output-projection row-tile + LayerNorm, interleaved
                    # into phase C: the PE has slack there (C is DVE-paced),
                    # so these matmuls hide in its gaps.  Borrows a scores
                    # PSUM tile for the accumulation.
                    inv_d = 1.0 / DM
                    ps_o4 = sc_psum.tile([P, HPG, QB], F32, tag="sc")
                    ps_o = ps_o4.rearrange("p h q -> p (h q)")
                    for kc in range(DT):          # stationary shared by 2 mms
                        for half in range(2):
                            nc.tensor.matmul(
                                ps_o[:, half * 512:(half + 1) * 512],
                                ctx_sb[:, kc, st * P:(st + 1) * P],
                                wfc_tiles[kc][:, half * 512:(half + 1) * 512],
                                start=(kc == 0), stop=(kc == DT - 1))
                    x_sb = ln_pool.tile([P, DM], F32, tag="x")
                    y_sb = ln_pool.tile([P, DM], BF16, tag="y")
                    bst = stat_pool.tile([P, 2, 6], F32, tag="bst")
                    mv = stat_pool.tile([P, 2], F32, tag="mv")
                    std = stat_pool.tile([P, 1], F32, tag="std")
                    rstd = stat_pool.tile([P, 1], F32, tag="rstd")
                    nb = stat_pool.tile([P, 1], F32, tag="nb")
                    # stats on DVE straight from PSUM, in parallel with the
                    # ACT eviction copy — drops the ACT Square pass + accums
                    # (bn_stats is capped at 512 free elems per call)
                    for ch in range(2):
                        nc.vector.bn_stats(bst[:, ch, :], ps_o4[:, ch, :])
                    nc.scalar.copy(x_sb[:], ps_o)
                    nc.vector.bn_aggr(mv[:], bst[:])
                    nc.scalar.activation(std[:], mv[:, 1:2], AF.Sqrt,
                                         bias=eps_t[:])
                    nc.vector.reciprocal(rstd[:], std[:])
                    nc.vector.scalar_tensor_tensor(nb[:], mv[:, 0:1], -1.0,
                                                   rstd[:], ALU.mult,
                                                   ALU.mult)
                    nc.scalar.activation(y_sb[:], x_sb[:], AF.Identity,
                                         bias=nb[:], scale=rstd[:])
                    if apply_gamma_beta:
                        nc.vector.tensor_mul(y_sb[:], y_sb[:], gm[:])
                        nc.vector.tensor_add(y_sb[:], y_sb[:], bt[:])
                    nc.sync.dma_start(out[st * P:(st + 1) * P, :], y_sb[:])
                def renorm_stages_dram(ctx4, qs, hg, slot):
                    # renorm via DRAM round-trip broadcast: DVE recip of the
                    # denom row -> DMA row to a DRAM scratch -> DMA back with
                    # a stride-0 source to partitions 0-63 (gamma pattern) ->
                    # fused PSUM*recip eviction muls on DVE.  Both DMAs ride
                    # the idle Pool queue; PE and ACT never enter the chain.
                    box = {}

                    def s_recip():
                        den = dn_pool.tile([P, HPG * QB], F32, tag="den")
                        nc.vector.reciprocal(den[64:65, :], ctx4[64:65, :])
                        box["den"] = den

                    def s_b1():
                        nc.gpsimd.dma_start(rbd[slot:slot + 1, :],
                                            box["den"][64:65, :])

                    def s_b2():
                        rb = dn_pool.tile([P, HPG * QB], F32, tag="rb")
                        d = rbd[slot:slot + 1, :]
                        nc.gpsimd.dma_start(
                            rb[0:64, :],
                            bass.AP(tensor=d.tensor, offset=d.offset,
                                    ap=[[0, 64]] + list(d.ap)[1:]))
                        box["rb"] = rb

                    def s_mul(hl):
                        h = hg * HPG + hl
                        jt, po = h // 2, (h % 2) * 64
                        cs = ctx4[0:64, hl * QB:(hl + 1) * QB]
                        rbs = box["rb"][0:64, hl * QB:(hl + 1) * QB]
                        if po == 0:
                            nc.vector.tensor_mul(ctx_sb[0:64, jt, qs],
                                                 cs, rbs)
                        else:
                            stg = stg_pool.tile([64, QB], BF16, tag="stg")
                            nc.vector.tensor_mul(stg[:], cs, rbs)
                            nc.sync.dma_start(ctx_sb[64:128, jt, qs], stg[:])

                    return {RN_SLOTS[0]: [s_recip], RN_SLOTS[1]: [s_b1],
                            RN_SLOTS[2]: [s_b2],
                            RN_SLOTS[3]: [lambda: s_mul(0)],
                            RN_SLOTS[4]: [lambda: s_mul(1)]}

                def renorm_stages(ctx4, qs, hg):
                    # renorm + evict as STAGED closures, fired at fixed kc
                    # slots inside the NEXT head-group's loop.  The chain is
                    # recip (DVE, f32r) -> rank-1 PE broadcast -> ACT copy ->
                    # fused PSUM*recip eviction muls on DVE; spreading the
                    # stages keeps each engine's in-order queue from stalling
                    # at its head while an upstream stage completes.  Odd
                    # heads still DMA-shift to partitions 64-127.
                    den_rs = []
                    box = {}

                    def s_recip():
                        for hl in range(HPG):
                            den_r = dn_pool.tile([P, QB], F32R, tag="denr",
                                                 name=f"denr{qs.start}{hg}{hl}")
                            with nc.allow_low_precision(
                                    reason="f32r recip feeds broadcast mm"):
                                nc.vector.reciprocal(
                                    den_r[64:65, :],
                                    ctx4[64:65, hl * QB:(hl + 1) * QB])
                            den_rs.append(den_r)

                    def s_rank1():
                        rb_ps = sc_psum.tile([P, HPG, QB], F32, tag="sc",
                                             name=f"rbps{qs.start}{hg}")
                        for hl in range(HPG):
                            nc.tensor.matmul(rb_ps[0:64, hl, :],
                                             r32(ones64[64:65, 0:64]),
                                             den_rs[hl][64:65, :])
                        box["rb_ps"] = rb_ps

                    def s_copy():
                        if not RN_PSUM_RB:
                            rb_sb = dn_pool.tile([P, HPG, QB], F32,
                                                 tag="rbsb")
                            nc.scalar.copy(rb_sb[0:64, :, :],
                                           box["rb_ps"][0:64, :, :])
                            box["rb_sb"] = rb_sb

                    def s_mul(hl):
                        h = hg * HPG + hl
                        jt, po = h // 2, (h % 2) * 64
                        cs = ctx4[0:64, hl * QB:(hl + 1) * QB]
                        if RN_PSUM_RB:
                            rbs = box["rb_ps"][0:64, hl, :]
                        else:
                            rbs = box["rb_sb"][0:64, hl, :]
                        if po == 0:
                            nc.vector.tensor_mul(ctx_sb[0:64, jt, qs],
                                                 cs, rbs)
                        else:
                            stg = stg_pool.tile([64, QB], BF16, tag="stg")
                            nc.vector.tensor_mul(stg[:], cs, rbs)
                            # Pool queue: keeps the shift off the SP queue
                            # that streams mt/maskt/wfc
                            nc.gpsimd.dma_start(ctx_sb[64:128, jt, qs],
                                                stg[:])

                    return {RN_SLOTS[0]: [s_recip], RN_SLOTS[1]: [s_rank1],
                            RN_SLOTS[2]: [s_copy],
                            RN_SLOTS[3]: [lambda: s_mul(0)],
                            RN_SLOTS[4]: [lambda: s_mul(1)]}

                pending_stages = {}
                for qb in range(NQB):
                    qs = slice(qb * QB, (qb + 1) * QB)
                    mt_qb = m_pool.tile([P, ST, QB], BF16, tag="mtq")
                    nc.sync.dma_start(mt_qb[:], mt_t[:, :, qs])
                    for hg in range(H // HPG):
                        ctx4 = ctx_psum.tile([65, HPG * QB], F32, tag="ctx4")
                        # mask correction: masked entries hold exp(0)=1.  For
                        # SUB_HGS groups it is removed by an in-place bf16
                        # p4 += maskT on DVE (maskT = -mask, so masked
                        # entries become exactly 0, and the ones row of V'
                        # then sums the corrected denominator); for the rest
                        # a -maskT matmul accumulates into the same PSUM
                        # group.  Split tuned to balance PE and DVE.  The
                        # add+AV of k-tile kc are emitted one tile late so
                        # the add never sits between exp and AV on the
                        # critical path.
                        use_sub = hg >= (H // HPG - SUB_HGS)
                        stages = pending_stages
                        pending_stages = {}

                        ab_mul = "nomul" in ablate
                        ab_exp = "noexp" in ablate
                        ab_mask = "nomask" in ablate
                        ab_av = "noav" in ablate
                        ab_scores = "noscores" in ablate

                        def emit_av(p4, kc, hg=hg, ctx4=ctx4, qs=qs):
                            for hl in range(HPG):
                                h = hg * HPG + hl
                                nc.tensor.matmul(
                                    ctx4[:, hl * QB:(hl + 1) * QB],
                                    vp_view[:, kc, h, 0:65],
                                    p4[:, hl, :],
                                    start=(ab_mask and kc == 0),
                                    stop=(kc == ST - 1))

                        def emit_add_av(p4, kc, hg=hg, ctx4=ctx4, qs=qs):
                            nc.vector.tensor_add(
                                p4[:], p4[:],
                                maskt_sb[:, kc:kc + 1, qs].broadcast_to(
                                    [P, HPG, QB]))
                            for hl in range(HPG):
                                h = hg * HPG + hl
                                nc.tensor.matmul(
                                    ctx4[:, hl * QB:(hl + 1) * QB],
                                    vp_view[:, kc, h, 0:65],
                                    p4[:, hl, :],
                                    start=(kc == 0), stop=(kc == ST - 1))

                        pend_av = None
                        av_q = []
                        t4p = None
                        paired = EXP_PAIR and not ablate and not use_sub
                        for kc in range(ST):
                            ps_s = sc_psum.tile([P, HPG, QB], F32, tag="sc")
                            if not ab_scores:
                                for hl in range(HPG):
                                    h = hg * HPG + hl
                                    jt, po = h // 2, (h % 2) * 64
                                    nc.tensor.matmul(
                                        ps_s[:, hl, :],
                                        kt_sb[po:po + 64, jt,
                                              kc * P:(kc + 1) * P],
                                        qt_sb[po:po + 64, jt, qs])
                            # one DVE op covers both heads; mt broadcast via
                            # stride-0 middle axis
                            if paired:
                                # one ACT exp per TWO k-tiles: halves ACT op
                                # count (352-cycle fixed cost each) and sems
                                half = kc % 2
                                if half == 0:
                                    t4p = att_pool.tile([P, 2 * HPG, QB],
                                                        F32, tag="t4")
                                nc.vector.tensor_mul(
                                    t4p[:, half * HPG:(half + 1) * HPG, :],
                                    ps_s[:],
                                    mt_qb[:, kc:kc + 1, :].broadcast_to(
                                        [P, HPG, QB]))
                                if half == 1:
                                    p4p = attp_pool.tile([P, 2 * HPG, QB],
                                                         BF16, tag="p4")
                                    nc.scalar.activation(p4p[:], t4p[:],
                                                         AF.Exp)
                                    av_q.append((p4p, kc - 1, 0))
                                    av_q.append((p4p, kc, 1))
                            else:
                                t4 = att_pool.tile([P, HPG, QB],
                                                   BF16 if ab_exp else F32,
                                                   tag="t4")
                                p4 = attp_pool.tile([P, HPG, QB], BF16,
                                                    tag="p4")
                                if not ab_mul:
                                    nc.vector.tensor_mul(
                                        t4[:], ps_s[:],
                                        mt_qb[:, kc:kc + 1, :].broadcast_to(
                                            [P, HPG, QB]))
                                if not ab_exp:
                                    nc.scalar.activation(
                                        p4[:], ps_s if ab_mul else t4[:],
                                        AF.Exp)
                                else:
                                    p4 = t4
                            for fn in stages.pop(kc, []):
                                fn()
                            if use_sub:
                                if pend_av is not None:
                                    emit_add_av(*pend_av)
                                pend_av = (p4, kc)
                                continue
                            # mask matmuls for THIS kc: independent of exp,
                            # so they never stall PE.  The p4 AV pair is
                            # emitted late so PE's in-order queue never parks
                            # on exp — otherwise scores(kc+1) can't issue
                            # until exp(kc) completes and the whole mul->exp
                            # pipeline serializes.
                            if not ab_mask:
                                for hl in range(HPG):
                                    h = hg * HPG + hl
                                    nc.tensor.matmul(
                                        ctx4[:, hl * QB:(hl + 1) * QB],
                                        vp_view[:, kc, h, 0:65],
                                        maskt_sb[:, kc, qs],
                                        start=(kc == 0),
                                        stop=(ab_av and kc == ST - 1))
                            if paired:
                                if av_q and av_q[0][1] <= kc - 2:
                                    p4p2, k2, j = av_q.pop(0)
                                    emit_av(
                                        p4p2[:, j * HPG:(j + 1) * HPG, :],
                                        k2)
                            elif not ab_av:
                                if pend_av is not None:
                                    emit_av(*pend_av)
                                pend_av = (p4, kc)
                        for p4p2, k2, j in av_q:
                            emit_av(p4p2[:, j * HPG:(j + 1) * HPG, :], k2)
                        if pend_av is not None:
                            emit_add_av(*pend_av) if use_sub else \
                                emit_av(*pend_av)
                        for kc in sorted(stages):   # unfired leftovers
                            for fn in stages.pop(kc):
                                fn()
                        if "norenorm" not in ablate:
                            if RENORM_MODE == "dram":
                                pending_stages = renorm_stages_dram(
                                    ctx4, qs, hg, (qb * 8 + hg) % 2)
                            else:
                                pending_stages = renorm_stages(ctx4, qs, hg)
                        # interleave the first half of phase D into the last
                        # q-block: qb0's rows are fully evicted once its
                        # final delayed renorm flushed (at qb1/hg0), so rows
                        # st=0..3 project+normalize inside qb1's PE slack
                        if D_ILV and qb == NQB - 1 and hg % 2 == 1:
                            emit_d_st((hg - 1) // 2)
                for kc in sorted(pending_stages):
                    for fn in pending_stages.pop(kc):
                        fn()
                if "D" in phases:
                    for st in range(0 if not D_ILV else ST // 2, ST):
                        emit_d_st(st)

            persist_cm.__exit__(None, None, None)

        if timing:
            with tc.tile_pool(name="donep", bufs=1) as dp:
                dt_ = dp.tile([1, 1], F32, tag="done")
                nc.vector.memset(dt_[:], 1.0)
                nc.sync.dma_start(done[:], dt_[:])

    nc.compile()
    return nc


_CACHE = {}


def _get_nc(apply_gamma_beta: bool):
    if apply_gamma_beta not in _CACHE:
        _CACHE[apply_gamma_beta] = build_bass(apply_gamma_beta)
    return _CACHE[apply_gamma_beta]


def _prep(inputs):
    """Build (nc, in_maps) for the SPMD run from the full unsharded inputs."""
    return _prep_impl(**inputs)


def _prep_impl(input_Q, input_K, input_V, attn_mask, matrix, Wq, Wk, Wv, Wfc,
               gamma, beta):
    input_Q = np.ascontiguousarray(np.asarray(input_Q, np.float32))
    input_K = np.ascontiguousarray(np.asarray(input_K, np.float32))
    input_V = np.ascontiguousarray(np.asarray(input_V, np.float32))
    attn_mask = np.asarray(attn_mask)
    matrix = np.asarray(matrix, np.float32)
    Wq = np.ascontiguousarray(np.asarray(Wq, np.float32))
    Wk = np.ascontiguousarray(np.asarray(Wk, np.float32))
    Wv = np.ascontiguousarray(np.asarray(Wv, np.float32))
    Wfc = np.ascontiguousarray(np.asarray(Wfc, np.float32))
    gamma = np.asarray(gamma, np.float32)
    beta = np.asarray(beta, np.float32)

    trivial_gb = bool(np.all(gamma == 1.0) and np.all(beta == 0.0))
    nc = _get_nc(not trivial_gb)

    from ml_dtypes import bfloat16
    wq_s = np.ascontiguousarray(Wq / np.sqrt(DK)).astype(bfloat16)
    wk_b = Wk.astype(bfloat16)
    wv_b = Wv.astype(bfloat16)
    wfc_b = Wfc.astype(bfloat16)
    keep = (~attn_mask).astype(np.float32)           # [B, S, S]
    m_eff = matrix[:, 0, :, :] * keep                # [B, S, S]

    in_maps = []
    for b in range(B):
        im = {
            "xq": input_Q[b].astype(bfloat16),
            "xk": input_K[b].astype(bfloat16),
            "xv": input_V[b].astype(bfloat16),
            "mt": np.ascontiguousarray(m_eff[b].T.astype(bfloat16)),
            "maskt": np.ascontiguousarray(-attn_mask[b].T.astype(bfloat16)),
            "wq": wq_s, "wk": wk_b, "wv": wv_b, "wfc": wfc_b,
            "ident": np.eye(P, dtype=bfloat16),
        }
        if not trivial_gb:
            im["gamma"] = gamma
            im["beta"] = beta
        in_maps.append(im)
    return nc, in_maps


def kernel(**inputs):
    nc, in_maps = _prep(inputs)
    res = run_bass_kernel_spmd(nc, in_maps, core_ids=list(range(B)))
    return np.stack([res.results[b]["out"] for b in range(B)],
                    axis=0).astype(np.float32)



# revision 82
# speedup vs baseline: 1.1663x; 1.1663x over previous
"""Trainium2 Bass kernel for MultiHeadAttention (B=8, S=1024, D=1024, H=16, DK=DV=64).

Sharding: data-parallel over batch — each of the 8 NeuronCores computes one
full batch element (QKV projections, masked+scaled softmax attention, output
projection, LayerNorm). No collectives.

Per-core math (batch b), matmul datapath in bf16 (f32 PSUM accumulate):
  Qt = (Wq/8)^T Xq^T          [hd, s]   (head-dim-major / transposed)
  Kt = Wk^T Xk^T              [hd, s]
  V' = Xv Wv (+ ones col)     [s, h*65]
  scores^T = K_h Q_h^T        [k, q] per head
  t = scores^T * mT           (mT = (matrix * !mask).T; masked entries -> 0)
  p = exp(t)                  (masked entries become exp(0) = 1)
  ctx^T = V'_h^T (p^T - maskT)  [65, q]  (row 64 = corrected denominator;
                               the -maskT matmul accumulates into the same
                               PSUM group and exactly removes the masked
                               exp(0)=1 contributions)
  renorm: DVE reciprocal of the denom row -> rank-1 PE broadcast matmul ->
          ACT copy -> fused PSUM*recip eviction multiply on DVE (odd heads
          DMA-shift to partitions 64-127 via the idle Pool queue).  The five
          renorm stages are emitted one head-group late at fixed kc slots
          (RN_SLOTS) of the next group so no engine queue parks at its head
          waiting on an upstream stage.
  out = LN(Ctx Wfc / denom) * gamma + beta
"""
from contextlib import ExitStack

import numpy as np

import concourse.bass as bass
import concourse.bacc as bacc
import concourse.tile as tile
import concourse.mybir as mybir
from concourse.bass_utils import run_bass_kernel_spmd
from concourse.masks import make_identity

F32 = mybir.dt.float32
F32R = mybir.dt.float32r
BF16 = mybir.dt.bfloat16
AF = mybir.ActivationFunctionType
ALU = mybir.AluOpType

B, S, DM, H, DK = 8, 1024, 1024, 16, 64
P = 128
ST = S // P      # seq tiles (8)
DT = DM // P     # d_model tiles (8)
QB = 512         # q-block width in attention phase
NQB = S // QB
HPG = 2          # heads per PSUM group
SUB_HGS = 0      # head-groups whose mask correction runs as an in-place
                 # DVE p4+=maskT (bf16 2x mode) instead of a PE matmul;
                 # tuned so PE and DVE loads balance
SC_BUFS = 2      # scores psum depth (each tile holds both heads: 2 banks)
CTX_BUFS = 2     # ctx psum depth
T4_BUFS = 4      # t4 depth (DVE->ACT hop)
P4_BUFS = 4      # p4 depth (ACT->PE hop)
D_ILV = False    # interleave first-half phase D into the last q-block
RN_SLOTS = (0, 1, 2, 5, 7)  # kc slots (in the next group) where the five
                 # renorm stages fire: recip, rank1, copy, mul0, mul1.
                 # Muls spread late at 5/7 beats 4/6 by ~5us; compressing
                 # everything to (0,1,2,3,4) loses ~12us and adjacent muls
                 # (6,7) lose ~9us — the DVE evict-muls must sit well clear
                 # of the ACT copy and of each other
RN_PSUM_RB = False  # eviction muls read the recip broadcast straight from
                 # PSUM (skip the ACT copy hop) — compiler rejects; keep False
RENORM_MODE = "pe"  # "pe": rank-1 PE broadcast + ACT copy;  "dram": recip
                 # row round-trips through a DRAM scratch and broadcasts back
                 # via a stride-0-source DMA (all off the PE/ACT queues)
EXP_PAIR = False  # one ACT exp per TWO k-tiles — measured slower (coarser
                 # exp granularity delays the AV accumulation); keep False
LN_EPS = 1e-5


def r32(ap):
    return ap.bitcast(F32R)


def build_bass(apply_gamma_beta: bool, timing_reps: int = 0, phases: str = "ABCD",
               ablate: str = ""):
    nc = bacc.Bacc("TRN2", target_bir_lowering=False, debug=False,
                   enable_asserts=False, num_devices=8)

    timing = timing_reps > 0
    kind = "Internal" if timing else "ExternalInput"

    def dram_in(name, shape, dt):
        if timing:
            return nc.dram_tensor(name, shape, dt).ap()
        return nc.dram_tensor(name, shape, dt, kind="ExternalInput").ap()

    xq = dram_in("xq", [S, DM], BF16)
    xk = dram_in("xk", [S, DM], BF16)
    xv = dram_in("xv", [S, DM], BF16)
    mt = dram_in("mt", [S, S], BF16)         # (matrix*keep)^T [k,q]
    maskt = dram_in("maskt", [S, S], BF16)   # mask^T as float [k,q]
    wq = dram_in("wq", [DM, DM], BF16)       # pre-scaled by 1/sqrt(DK)
    wk = dram_in("wk", [DM, DM], BF16)
    wv = dram_in("wv", [DM, DM], BF16)
    wfc = dram_in("wfc", [DM, DM], BF16)
    rbd = nc.dram_tensor("rbd", [2, HPG * QB], F32).ap()  # renorm scratch
    ident_d = dram_in("ident", [P, P], BF16)  # host-provided identity: avoids
                                              # gpsimd make_identity (~8us/op
                                              # dispatch on HW) at startup
    if timing:
        out = nc.dram_tensor("out", [S, DM], BF16).ap()
        done = nc.dram_tensor("done", [1, 1], F32, kind="ExternalOutput").ap()
    else:
        out = nc.dram_tensor("out", [S, DM], BF16,
                             kind="ExternalOutput").ap()
    gamma = beta = None
    if apply_gamma_beta:
        gamma = dram_in("gamma", [DM], F32)
        beta = dram_in("beta", [DM], F32)

    mt_t = mt.rearrange("(t p) q -> p t q", p=P)
    maskt_t = maskt.rearrange("(t p) q -> p t q", p=P)

    with tile.TileContext(nc) as tc, ExitStack() as ctx:
        if timing:
            ctx.enter_context(tc.For_i(0, timing_reps, 1))
        const = ctx.enter_context(tc.tile_pool(name="const", bufs=1))
        eps_t = const.tile([P, 1], F32, tag="eps")
        nc.vector.memset(eps_t[:], LN_EPS)

        # Ctx^T lives through C+D; opened first so A-C pools can release
        ctx_pool = ctx.enter_context(tc.tile_pool(name="ctxp", bufs=1))
        ctx_sb = ctx_pool.tile([P, DT, S], BF16, tag="ctx")       # Ctx^T [hd, q]
        if "norenorm" in ablate:
            nc.vector.memset(ctx_sb[:], 0.0)  # keep phase D's reads legal
        # Wfc tiles also outlive C: DMAs are issued at the start of phase C
        # so phase D's matmuls never wait on them
        wfc_pool = ctx.enter_context(tc.tile_pool(name="wfc", bufs=1))
        wfc_tiles = [wfc_pool.tile([P, DM], BF16, tag=f"wfc{kc}",
                                   name=f"wfct{kc}")
                     for kc in range(DT)]

        # persistent across phases A-C (released before phase D)
        persist_cm = tc.tile_pool(name="persist", bufs=1)
        persist = persist_cm.__enter__()
        qt_sb = persist.tile([P, DT, S], BF16, tag="qt")          # Qt [hd, s]
        kt_sb = persist.tile([P, DT, S], BF16, tag="kt")          # Kt [hd, s]
        vp_sb = persist.tile([P, ST, H * 65], BF16, tag="vp")     # V' [s, h*65]
        vp_view = vp_sb.rearrange("p t (h d) -> p t h d", d=65)

        # ---------- Phase A: transposes + QKV projections ----------
        if "A" in phases:
          with tc.tile_pool(name="xrow", bufs=6) as xrow_pool, \
             tc.tile_pool(name="xT", bufs=3) as xT_pool, \
             tc.tile_pool(name="wload", bufs=6) as w_pool, \
             tc.tile_pool(name="aconst", bufs=1) as aconst, \
             tc.tile_pool(name="tp_psum", bufs=4, space="PSUM") as tp_psum, \
             tc.tile_pool(name="pj_psum", bufs=4, space="PSUM") as pj_psum:

            ident = aconst.tile([P, P], BF16, tag="ident")
            nc.sync.dma_start(ident[:], ident_d)

            def transpose_input(x_ap):
                """DRAM x [S, DM] -> SBUF x^T [P, DT, S] (partition=dm, free=s).

                Evictions ride DVE (idle in phase A; 2x bf16 mode) so ACT
                never sits between a transpose and the projection that
                consumes it."""
                xT = xT_pool.tile([P, DT, S], BF16, tag="xT")
                for i in range(ST):               # source s-tile
                    xrow = xrow_pool.tile([P, DM], BF16, tag="xrow")
                    nc.sync.dma_start(xrow[:], x_ap[i * P:(i + 1) * P, :])
                    for j0 in range(0, DT, 4):    # 4 dm-tiles per psum bank
                        ps = tp_psum.tile([P, 4, P], BF16, tag="tp")
                        for jj in range(4):
                            nc.tensor.matmul(ps[:, jj, :],
                                             xrow[:, (j0 + jj) * P:(j0 + jj + 1) * P],
                                             ident[:], is_transpose=True)
                        # strided evict: ps [P,4,P] -> xT[:, j0:j0+4, i*P:(i+1)*P]
                        nc.vector.tensor_copy(
                            xT[:, j0:j0 + 4, i * P:(i + 1) * P], ps[:])
                return xT

            def load_w_half(w_ap, half):
                """Stream one column-half of a weight matrix: [P, DT, DM/2]."""
                w_sb = w_pool.tile([P, DT, DM // 2], BF16, tag="w")
                nc.sync.dma_start(
                    w_sb[:],
                    w_ap.rearrange("(t p) n -> p t n", p=P)[
                        :, :, half * (DM // 2):(half + 1) * (DM // 2)])
                return w_sb

            def proj_T(w_sbs, xT, dst):
                """dst[hd, s] = W^T X^T : lhsT = W tiles [dm, hd], rhs = X^T [dm, s]."""
                for wh in range(2):               # W column halves
                    w_sb = w_sbs[wh]
                    for jm2 in range(DT // 2):    # hd out tiles in this half
                        jm = wh * (DT // 2) + jm2
                        pss = [pj_psum.tile([P, 512], F32, tag="pj",
                                            name=f"pj{jm}{sn}")
                               for sn in range(2)]
                        for kc in range(DT):      # stationary shared by 2 mms
                            for sn in range(2):
                                nc.tensor.matmul(
                                    pss[sn][:],
                                    w_sb[:, kc, jm2 * P:(jm2 + 1) * P],
                                    xT[:, kc, sn * 512:(sn + 1) * 512],
                                    start=(kc == 0), stop=(kc == DT - 1))
                        for sn in range(2):
                            # alternate evict engines: ACT and DVE both have
                            # slack under the PE-bound projections
                            ev = nc.scalar.copy if sn == 0 else \
                                nc.vector.tensor_copy
                            ev(dst[:, jm, sn * 512:(sn + 1) * 512],
                               pss[sn][:])

            # all three transposes are emitted first (xT triple-buffered):
            # the first projection then never bubbles on its own eviction
            # tail, and the weight DMAs stream under the transposes
            xkT = transpose_input(xk)
            wk_sbs = [load_w_half(wk, wh) for wh in range(2)]
            xqT = transpose_input(xq)
            wq_sbs = [load_w_half(wq, wh) for wh in range(2)]
            xvT = transpose_input(xv)
            wv_sbs = [load_w_half(wv, wh) for wh in range(2)]
            proj_T(wk_sbs, xkT, kt_sb)
            proj_T(wq_sbs, xqT, qt_sb)

            # V projection: natural [s, hd]; lhsT = Xv^T tiles, rhs = Wv halves
            for jm in range(ST):                  # s out tile
                pss = [pj_psum.tile([P, 512], F32, tag="pj", name=f"pv{jm}{wh}")
                       for wh in range(2)]
                for kc in range(DT):              # stationary shared by 2 mms
                    for wh in range(2):
                        nc.tensor.matmul(
                            pss[wh][:],
                            xvT[:, kc, jm * P:(jm + 1) * P],
                            wv_sbs[wh][:, kc, :],
                            start=(kc == 0), stop=(kc == DT - 1))
                for wh in range(2):
                    ev = nc.scalar.copy if wh == 0 else nc.vector.tensor_copy
                    ev(vp_view[:, jm, wh * 8:(wh + 1) * 8, 0:64],
                       pss[wh].rearrange("p (h d) -> p h d", d=64))
            nc.vector.memset(vp_view[:, :, :, 64:65], 1.0)

        if True:

            # ---------- Phase C: attention ----------
            if "C" in phases:
              with tc.tile_pool(name="mstream", bufs=1) as m_pool, \
                 tc.tile_pool(name="mask", bufs=1) as mask_pool, \
                 tc.tile_pool(name="att", bufs=T4_BUFS) as att_pool, \
                 tc.tile_pool(name="attp", bufs=P4_BUFS) as attp_pool, \
                 tc.tile_pool(name="cconst", bufs=1) as cconst, \
                 tc.tile_pool(name="rbp", bufs=2) as stg_pool, \
                 tc.tile_pool(name="dn", bufs=2) as dn_pool, \
                 tc.tile_pool(name="ln", bufs=2) as ln_pool, \
                 tc.tile_pool(name="lnstat", bufs=4) as stat_pool, \
                 tc.tile_pool(name="gb", bufs=1) as gb_pool, \
                 tc.tile_pool(name="sc_psum", bufs=SC_BUFS, space="PSUM") as sc_psum, \
                 tc.tile_pool(name="ctx_psum", bufs=CTX_BUFS, space="PSUM") as ctx_psum:
                maskt_sb = mask_pool.tile([P, ST, S], BF16, tag="maskt")
                nc.sync.dma_start(maskt_sb[:], maskt_t)
                if "D" in phases:
                    for kc in range(DT):
                        nc.sync.dma_start(wfc_tiles[kc][:],
                                          wfc[kc * P:(kc + 1) * P, :])
                # all-ones row at partition 64: stationary for the denominator
                # broadcast matmuls (rank-1 outer product with the recip row)
                ones64 = cconst.tile([P, P], F32, tag="ones64")
                nc.vector.memset(ones64[64:65, :], 1.0)
                gm = bt = None
                if apply_gamma_beta:
                    gm = gb_pool.tile([P, DM], F32, tag="gm")
                    bt = gb_pool.tile([P, DM], F32, tag="bt")
                    nc.sync.dma_start(gm[:], bass.AP(
                        tensor=gamma.tensor, offset=gamma.offset,
                        ap=[[0, P]] + list(gamma.ap)))
                    nc.sync.dma_start(bt[:], bass.AP(
                        tensor=beta.tensor, offset=beta.offset,
                        ap=[[0, P]] + list(beta.ap)))

                def emit_d_st(st):
                    # one output-projection row-tile + LayerNorm, interleaved
                    # into phase C: the PE has slack there (C is DVE-paced),
                    # so these matmuls hide in its gaps.  Borrows a scores
                    # PSUM tile for the accumulation.
                    inv_d = 1.0 / DM
                    ps_o4 = sc_psum.tile([P, HPG, QB], F32, tag="sc")
                    ps_o = ps_o4.rearrange("p h q -> p (h q)")
                    for kc in range(DT):          # stationary shared by 2 mms
                        for half in range(2):
                            nc.tensor.matmul(
                                ps_o[:, half * 512:(half + 1) * 512],
                                ctx_sb[:, kc, st * P:(st + 1) * P],
                                wfc_tiles[kc][:, half * 512:(half + 1) * 512],
                                start=(kc == 0), stop=(kc == DT - 1))
                    x_sb = ln_pool.tile([P, DM], F32, tag="x")
                    y_sb = ln_pool.tile([P, DM], BF16, tag="y")
                    bst = stat_pool.tile([P, 2, 6], F32, tag="bst")
                    mv = stat_pool.tile([P, 2], F32, tag="mv")
                    std = stat_pool.tile([P, 1], F32, tag="std")
                    rstd = stat_pool.tile([P, 1], F32, tag="rstd")
                    nb = stat_pool.tile([P, 1], F32, tag="nb")
                    # stats on DVE straight from PSUM, in parallel with the
                    # ACT eviction copy — drops the ACT Square pass + accums
                    # (bn_stats is capped at 512 free elems per call)
                    for ch in range(2):
                        nc.vector.bn_stats(bst[:, ch, :], ps_o4[:, ch, :])
                    nc.scalar.copy(x_sb[:], ps_o)
                    nc.vector.bn_aggr(mv[:], bst[:])
                    nc.scalar.activation(std[:], mv[:, 1:2], AF.Sqrt,
                                         bias=eps_t[:])
                    nc.vector.reciprocal(rstd[:], std[:])
                    nc.vector.scalar_tensor_tensor(nb[:], mv[:, 0:1], -1.0,
                                                   rstd[:], ALU.mult,
                                                   ALU.mult)
                    nc.scalar.activation(y_sb[:], x_sb[:], AF.Identity,
                                         bias=nb[:], scale=rstd[:])
                    if apply_gamma_beta:
                        nc.vector.tensor_mul(y_sb[:], y_sb[:], gm[:])
                        nc.vector.tensor_add(y_sb[:], y_sb[:], bt[:])
                    nc.sync.dma_start(out[st * P:(st + 1) * P, :], y_sb[:])
                def renorm_stages_dram(ctx4, qs, hg, slot):
                    # renorm via DRAM round-trip broadcast: DVE recip of the
                    # denom row -> DMA row to a DRAM scratch -> DMA back with
                    # a stride-0 source to partitions 0-63 (gamma pattern) ->
                    # fused PSUM*recip eviction muls on DVE.  Both DMAs ride
                    # the idle Pool queue; PE and ACT never enter the chain.
                    box = {}

                    def s_recip():
                        den = dn_pool.tile([P, HPG * QB], F32, tag="den")
                        nc.vector.reciprocal(den[64:65, :], ctx4[64:65, :])
                        box["den"] = den

                    def s_b1():
                        nc.gpsimd.dma_start(rbd[slot:slot + 1, :],
                                            box["den"][64:65, :])

                    def s_b2():
                        rb = dn_pool.tile([P, HPG * QB], F32, tag="rb")
                        d = rbd[slot:slot + 1, :]
                        nc.gpsimd.dma_start(
                            rb[0:64, :],
                            bass.AP(tensor=d.tensor, offset=d.offset,
                                    ap=[[0, 64]] + list(d.ap)[1:]))
                        box["rb"] = rb

                    def s_mul(hl):
                        h = hg * HPG + hl
                        jt, po = h // 2, (h % 2) * 64
                        cs = ctx4[0:64, hl * QB:(hl + 1) * QB]
                        rbs = box["rb"][0:64, hl * QB:(hl + 1) * QB]
                        if po == 0:
                            nc.vector.tensor_mul(ctx_sb[0:64, jt, qs],
                                                 cs, rbs)
                        else:
                            stg = stg_pool.tile([64, QB], BF16, tag="stg")
                            nc.vector.tensor_mul(stg[:], cs, rbs)
                            nc.sync.dma_start(ctx_sb[64:128, jt, qs], stg[:])

                    return {RN_SLOTS[0]: [s_recip], RN_SLOTS[1]: [s_b1],
                            RN_SLOTS[2]: [s_b2],
                            RN_SLOTS[3]: [lambda: s_mul(0)],
                            RN_SLOTS[4]: [lambda: s_mul(1)]}

                def renorm_stages(ctx4, qs, hg):
                    # renorm + evict as STAGED closures, fired at fixed kc
                    # slots inside the NEXT head-group's loop.  The chain is
                    # recip (DVE, f32r) -> rank-1 PE broadcast -> ACT copy ->
                    # fused PSUM*recip eviction muls on DVE; spreading the
                    # stages keeps each engine's in-order queue from stalling
                    # at its head while an upstream stage completes.  Odd
                    # heads still DMA-shift to partitions 64-127.
                    den_rs = []
                    box = {}

                    def s_recip():
                        for hl in range(HPG):
                            den_r = dn_pool.tile([P, QB], F32R, tag="denr",
                                                 name=f"denr{qs.start}{hg}{hl}")
                            with nc.allow_low_precision(
                                    reason="f32r recip feeds broadcast mm"):
                                nc.vector.reciprocal(
                                    den_r[64:65, :],
                                    ctx4[64:65, hl * QB:(hl + 1) * QB])
                            den_rs.append(den_r)

                    def s_rank1():
                        rb_ps = sc_psum.tile([P, HPG, QB], F32, tag="sc",
                                             name=f"rbps{qs.start}{hg}")
                        for hl in range(HPG):
                            nc.tensor.matmul(rb_ps[0:64, hl, :],
                                             r32(ones64[64:65, 0:64]),
                                             den_rs[hl][64:65, :])
                        box["rb_ps"] = rb_ps

                    def s_copy():
                        if not RN_PSUM_RB:
                            rb_sb = dn_pool.tile([P, HPG, QB], F32,
                                                 tag="rbsb")
                            nc.scalar.copy(rb_sb[0:64, :, :],
                                           box["rb_ps"][0:64, :, :])
                            box["rb_sb"] = rb_sb

                    def s_mul(hl):
                        h = hg * HPG + hl
                        jt, po = h // 2, (h % 2) * 64
                        cs = ctx4[0:64, hl * QB:(hl + 1) * QB]
                        if RN_PSUM_RB:
                            rbs = box["rb_ps"][0:64, hl, :]
                        else:
                            rbs = box["rb_sb"][0:64, hl, :]
                        if po == 0:
                            nc.vector.tensor_mul(ctx_sb[0:64, jt, qs],
                                                 cs, rbs)
                        else:
                            stg = stg_pool.tile([64, QB], BF16, tag="stg")
                            nc.vector.tensor_mul(stg[:], cs, rbs)
                            # Pool queue: keeps the shift off the SP queue
                            # that streams mt/maskt/wfc
                            nc.gpsimd.dma_start(ctx_sb[64:128, jt, qs],
                                                stg[:])

                    return {RN_SLOTS[0]: [s_recip], RN_SLOTS[1]: [s_rank1],
                            RN_SLOTS[2]: [s_copy],
                            RN_SLOTS[3]: [lambda: s_mul(0)],
                            RN_SLOTS[4]: [lambda: s_mul(1)]}

                pending_stages = {}
                for qb in range(NQB):
                    qs = slice(qb * QB, (qb + 1) * QB)
                    mt_qb = m_pool.tile([P, ST, QB], BF16, tag="mtq")
                    nc.sync.dma_start(mt_qb[:], mt_t[:, :, qs])
                    for hg in range(H // HPG):
                        ctx4 = ctx_psum.tile([65, HPG * QB], F32, tag="ctx4")
                        # mask correction: masked entries hold exp(0)=1.  For
                        # SUB_HGS groups it is removed by an in-place bf16
                        # p4 += maskT on DVE (maskT = -mask, so masked
                        # entries become exactly 0, and the ones row of V'
                        # then sums the corrected denominator); for the rest
                        # a -maskT matmul accumulates into the same PSUM
                        # group.  Split tuned to balance PE and DVE.  The
                        # add+AV of k-tile kc are emitted one tile late so
                        # the add never sits between exp and AV on the
                        # critical path.
                        use_sub = hg >= (H // HPG - SUB_HGS)
                        stages = pending_stages
                        pending_stages = {}

                        ab_mul = "nomul" in ablate
                        ab_exp = "noexp" in ablate
                        ab_mask = "nomask" in ablate
                        ab_av = "noav" in ablate
                        ab_scores = "noscores" in ablate

                        def emit_av(p4, kc, hg=hg, ctx4=ctx4, qs=qs):
                            for hl in range(HPG):
                                h = hg * HPG + hl
                                nc.tensor.matmul(
                                    ctx4[:, hl * QB:(hl + 1) * QB],
                                    vp_view[:, kc, h, 0:65],
                                    p4[:, hl, :],
                                    start=(ab_mask and kc == 0),
                                    stop=(kc == ST - 1))

                        def emit_add_av(p4, kc, hg=hg, ctx4=ctx4, qs=qs):
                            nc.vector.tensor_add(
                                p4[:], p4[:],
                                maskt_sb[:, kc:kc + 1, qs].broadcast_to(
                                    [P, HPG, QB]))
                            for hl in range(HPG):
                                h = hg * HPG + hl
                                nc.tensor.matmul(
                                    ctx4[:, hl * QB:(hl + 1) * QB],
                                    vp_view[:, kc, h, 0:65],
                                    p4[:, hl, :],
                                    start=(kc == 0), stop=(kc == ST - 1))

                        pend_av = None
                        av_q = []
                        t4p = None
                        paired = EXP_PAIR and not ablate and not use_sub
                        for kc in range(ST):
                            ps_s = sc_psum.tile([P, HPG, QB], F32, tag="sc")
                            if not ab_scores:
                                for hl in range(HPG):
                                    h = hg * HPG + hl
                                    jt, po = h // 2, (h % 2) * 64
                                    nc.tensor.matmul(
                                        ps_s[:, hl, :],
                                        kt_sb[po:po + 64, jt,
                                              kc * P:(kc + 1) * P],
                                        qt_sb[po:po + 64, jt, qs])
                            # one DVE op covers both heads; mt broadcast via
                            # stride-0 middle axis
                            if paired:
                                # one ACT exp per TWO k-tiles: halves ACT op
                                # count (352-cycle fixed cost each) and sems
                                half = kc % 2
                                if half == 0:
                                    t4p = att_pool.tile([P, 2 * HPG, QB],
                                                        F32, tag="t4")
                                nc.vector.tensor_mul(
                                    t4p[:, half * HPG:(half + 1) * HPG, :],
                                    ps_s[:],
                                    mt_qb[:, kc:kc + 1, :].broadcast_to(
                                        [P, HPG, QB]))
                                if half == 1:
                                    p4p = attp_pool.tile([P, 2 * HPG, QB],
                                                         BF16, tag="p4")
                                    nc.scalar.activation(p4p[:], t4p[:],
                                                         AF.Exp)
                                    av_q.append((p4p, kc - 1, 0))
                                    av_q.append((p4p, kc, 1))
                            else:
                                t4 = att_pool.tile([P, HPG, QB],
                                                   BF16 if ab_exp else F32,
                                                   tag="t4")
                                p4 = attp_pool.tile([P, HPG, QB], BF16,
                                                    tag="p4")
                                if not ab_mul:
                                    nc.vector.tensor_mul(
                                        t4[:], ps_s[:],
                                        mt_qb[:, kc:kc + 1, :].broadcast_to(
                                            [P, HPG, QB]))
                                if not ab_exp:
                                    nc.scalar.activation(
                                        p4[:], ps_s if ab_mul else t4[:],
                                        AF.Exp)
                                else:
                                    p4 = t4
                            for fn in stages.pop(kc, []):
                                fn()
                            if use_sub:
                                if pend_av is not None:
                                    emit_add_av(*pend_av)
                                pend_av = (p4, kc)
                                continue
                            # mask matmuls for THIS kc: independent of exp,
                            # so they never stall PE.  The p4 AV pair is
                            # emitted late so PE's in-order queue never parks
                            # on exp — otherwise scores(kc+1) can't issue
                            # until exp(kc) completes and the whole mul->exp
                            # pipeline serializes.
                            if not ab_mask:
                                for hl in range(HPG):
                                    h = hg * HPG + hl
                                    nc.tensor.matmul(
                                        ctx4[:, hl * QB:(hl + 1) * QB],
                                        vp_view[:, kc, h, 0:65],
                                        maskt_sb[:, kc, qs],
                                        start=(kc == 0),
                                        stop=(ab_av and kc == ST - 1))
                            if paired:
                                if av_q and av_q[0][1] <= kc - 2:
                                    p4p2, k2, j = av_q.pop(0)
                                    emit_av(
                                        p4p2[:, j * HPG:(j + 1) * HPG, :],
                                        k2)
                            elif not ab_av:
                                if pend_av is not None:
                                    emit_av(*pend_av)
                                pend_av = (p4, kc)
                        for p4p2, k2, j in av_q:
                            emit_av(p4p2[:, j * HPG:(j + 1) * HPG, :], k2)
                        if pend_av is not None:
                            emit_add_av(*pend_av) if use_sub else \
                                emit_av(*pend_av)
                        for kc in sorted(stages):   # unfired leftovers
                            for fn in stages.pop(kc):
                                fn()
                        if "norenorm" not in ablate:
                            if RENORM_MODE == "dram":
                                pending_stages = renorm_stages_dram(
                                    ctx4, qs, hg, (qb * 8 + hg) % 2)
                            else:
                                pending_stages = renorm_stages(ctx4, qs, hg)
                        # interleave the first half of phase D into the last
                        # q-block: qb0's rows are fully evicted once its
                        # final delayed renorm flushed (at qb1/hg0), so rows
                        # st=0..3 project+normalize inside qb1's PE slack
                        if D_ILV and qb == NQB - 1 and hg % 2 == 1:
                            emit_d_st((hg - 1) // 2)
                for kc in sorted(pending_stages):
                    for fn in pending_stages.pop(kc):
                        fn()
                if "D" in phases:
                    for st in range(0 if not D_ILV else ST // 2, ST):
                        emit_d_st(st)

            persist_cm.__exit__(None, None, None)

        if timing:
            with tc.tile_pool(name="donep", bufs=1) as dp:
                dt_ = dp.tile([1, 1], F32, tag="done")
                nc.vector.memset(dt_[:], 1.0)
                nc.sync.dma_start(done[:], dt_[:])

    nc.compile()
    return nc


_CACHE = {}


def _get_nc(apply_gamma_beta: bool):
    if apply_gamma_beta not in _CACHE:
        _CACHE[apply_gamma_beta] = build_bass(apply_gamma_beta)
    return _CACHE[apply_gamma_beta]


def _prep(inputs):
    """Build (nc, in_maps) for the SPMD run from the full unsharded inputs."""
    return _prep_impl(**inputs)


def _prep_impl(input_Q, input_K, input_V, attn_mask, matrix, Wq, Wk, Wv, Wfc,
               gamma, beta):
    input_Q = np.ascontiguousarray(np.asarray(input_Q, np.float32))
    input_K = np.ascontiguousarray(np.asarray(input_K, np.float32))
    input_V = np.ascontiguousarray(np.asarray(input_V, np.float32))
    attn_mask = np.asarray(attn_mask)
    matrix = np.asarray(matrix, np.float32)
    Wq = np.ascontiguousarray(np.asarray(Wq, np.float32))
    Wk = np.ascontiguousarray(np.asarray(Wk, np.float32))
    Wv = np.ascontiguousarray(np.asarray(Wv, np.float32))
    Wfc = np.ascontiguousarray(np.asarray(Wfc, np.float32))
    gamma = np.asarray(gamma, np.float32)
    beta = np.asarray(beta, np.float32)

    trivial_gb = bool(np.all(gamma == 1.0) and np.all(beta == 0.0))
    nc = _get_nc(not trivial_gb)

    from ml_dtypes import bfloat16
    wq_s = np.ascontiguousarray(Wq / np.sqrt(DK)).astype(bfloat16)
    wk_b = Wk.astype(bfloat16)
    wv_b = Wv.astype(bfloat16)
    wfc_b = Wfc.astype(bfloat16)
    keep = (~attn_mask).astype(np.float32)           # [B, S, S]
    m_eff = matrix[:, 0, :, :] * keep                # [B, S, S]

    in_maps = []
    for b in range(B):
        im = {
            "xq": input_Q[b].astype(bfloat16),
            "xk": input_K[b].astype(bfloat16),
            "xv": input_V[b].astype(bfloat16),
            "mt": np.ascontiguousarray(m_eff[b].T.astype(bfloat16)),
            "maskt": np.ascontiguousarray(-attn_mask[b].T.astype(bfloat16)),
            "wq": wq_s, "wk": wk_b, "wv": wv_b, "wfc": wfc_b,
            "ident": np.eye(P, dtype=bfloat16),
        }
        if not trivial_gb:
            im["gamma"] = gamma
            im["beta"] = beta
        in_maps.append(im)
    return nc, in_maps


def kernel(**inputs):
    nc, in_maps = _prep(inputs)
    res = run_bass_kernel_spmd(nc, in_maps, core_ids=list(range(B)))
    return np.stack([res.results[b]["out"] for b in range(B)],
                    axis=0).astype(np.float32)

